# revision 1
# baseline (speedup 1.0000x reference)
"""Trainium2 Bass kernel for nn_Attention_36799279792519.

Full causal self-attention layer (QKV proj + RoPE + causal softmax attention +
output proj), B=2 T=2048 C=1024 H=16 D=64, sharded over 8 NeuronCores:
data-parallel on batch (2) x tensor-parallel on heads (4 heads/core).
Each core computes its heads' attention output and a partial projection
(T, C); the host sums the 4 partials per batch and adds proj bias.

Device-side layouts (per core):
  xT      (C, T)  bf16      hidden transposed (C on partitions)
  q/k     computed in (r, T) "ab-layout": rows 0-127 = the 4 heads' even
          dims stacked, rows 128-255 = odd dims (host permutes weight rows),
          so RoPE is 6 full-width DVE ops; then SBUF->SBUF DMAs permute to
          per-head-contiguous (64 rows per head).
  scores  computed transposed: sT (k, t) blocks, exp'd with no max
          subtraction (inputs are bounded), diagonal masked multiplicatively
          post-exp with a 0/1 tile.
  V       (t, r) with a ones column per head (65 cols/head) so the PV matmul
          also accumulates the softmax denominator as output row 64.
  out     oT (d_local, T), normalized during PSUM evacuation, then projected.

Self-contained: hardcodes all shapes; no sibling imports.
"""
import contextlib

import numpy as np
import ml_dtypes

import concourse.bass as bass
import concourse.mybir as mybir
import concourse.tile as tile
from concourse import bacc
from concourse.bass_utils import run_bass_kernel_spmd

B, T, C = 2, 2048, 1024
H, D = 16, 64
SCALE = D ** -0.5
NCORES = 8
CORES_PER_B = NCORES // B          # 4
HPC = H // CORES_PER_B             # 4 heads per core
RL = HPC * D                       # 256 local q/k/v rows
CCH = C // 128                     # 8 contraction chunks
TCH = T // 128                     # 16 t chunks of 128
NT = T // 512                      # 4 t chunks of 512
KB = T // 128                      # 16 key blocks of 128

F32 = mybir.dt.float32
BF16 = mybir.dt.bfloat16
BF = ml_dtypes.bfloat16

_compiled = {}


def _build():
    nc = bacc.Bacc("TRN2", target_bir_lowering=False, debug=False,
                   num_devices=NCORES)

    d = {}
    d["xT"] = nc.dram_tensor("xT", [C, T], BF16, kind="ExternalInput").ap()
    d["wq"] = nc.dram_tensor("wq_t", [C, RL], BF16, kind="ExternalInput").ap()
    d["wk"] = nc.dram_tensor("wk_t", [C, RL], BF16, kind="ExternalInput").ap()
    d["wv"] = nc.dram_tensor("wv_t", [C, RL], BF16, kind="ExternalInput").ap()
    d["wp"] = nc.dram_tensor("wproj_t", [RL, C], BF16, kind="ExternalInput").ap()
    d["bq"] = nc.dram_tensor("bq", [RL], F32, kind="ExternalInput").ap()
    d["bk"] = nc.dram_tensor("bk", [RL], F32, kind="ExternalInput").ap()
    d["bv"] = nc.dram_tensor("bv", [RL], F32, kind="ExternalInput").ap()
    d["ck"] = nc.dram_tensor("cos_k", [128, T], BF16, kind="ExternalInput").ap()
    d["sk"] = nc.dram_tensor("sin_k", [128, T], BF16, kind="ExternalInput").ap()
    d["mask"] = nc.dram_tensor("mask01", [128, 128], BF16,
                               kind="ExternalInput").ap()
    d["out"] = nc.dram_tensor("out", [T, C], BF16, kind="ExternalOutput").ap()
    d["recs"] = nc.dram_tensor("rec_scratch", [HPC * NT, 512], F32).ap()

    with tile.TileContext(nc) as tc:
        _program(nc, tc, d)

    nc.compile()
    return nc


def _program(nc, tc, d):
    with (
        tc.tile_pool(name="const", bufs=1) as const,
        tc.tile_pool(name="qk", bufs=1) as qkpool,
        tc.tile_pool(name="small", bufs=2) as spool,
        tc.tile_pool(name="outsb", bufs=8) as opool,
    ):
        # ================= long-lived tiles =================
        wp_sb = const.tile([128, 2, C], BF16)
        mask_sb = const.tile([128, 128], BF16)

        # long-lived products of phase 1
        v_sb = qkpool.tile([128, TCH, HPC, 65], BF16, tag="v")
        nc.vector.memset(v_sb[:, :, :, 64:65], 1.0)
        qr = [qkpool.tile([128, T], BF16, tag=f"qrh{i}", name=f"qrh{i}") for i in range(2)]
        kr = [qkpool.tile([128, T], BF16, tag=f"krh{i}", name=f"krh{i}") for i in range(2)]

        # ============ phase 1 (scoped pool): QKV + RoPE + permute ============
        with (
            tc.tile_pool(name="ph1", bufs=1) as p1,
            tc.tile_pool(name="ps_qkv", bufs=6, space="PSUM") as ps_qkv,
        ):
            # tiny exp to pull the ACT table load into the DMA window
            warm = p1.tile([128, 1], F32, tag="warm", name="warm")
            nc.vector.memset(warm, 0.0)
            nc.scalar.activation(out=warm, in_=warm,
                                 func=mybir.ActivationFunctionType.Exp)
            wq_sb = p1.tile([128, CCH, RL], BF16)
            wq_r = d["wq"].rearrange("(cc p) r -> p cc r", p=128)
            x_sb = p1.tile([128, CCH, T], BF16)           # (128, 8, 2048)
            xT_r = d["xT"].rearrange("(cc p) t -> p cc t", p=128)
            for cc in range(CCH):
                nc.sync.dma_start(out=wq_sb[:, cc, :], in_=wq_r[:, cc, :])
                nc.sync.dma_start(out=x_sb[:, cc, :], in_=xT_r[:, cc, :])
            wk_sb = p1.tile([128, CCH, RL], BF16)
            nc.sync.dma_start(out=wk_sb,
                              in_=d["wk"].rearrange("(cc p) r -> p cc r", p=128))
            wv_sb = p1.tile([128, CCH, RL], BF16)
            nc.sync.dma_start(out=wv_sb,
                              in_=d["wv"].rearrange("(cc p) r -> p cc r", p=128))
            bq_sb = p1.tile([128, 2], F32)
            nc.sync.dma_start(out=bq_sb,
                              in_=d["bq"].rearrange("(rc p) -> p rc", p=128))
            bk_sb = p1.tile([128, 2], F32)
            nc.sync.dma_start(out=bk_sb,
                              in_=d["bk"].rearrange("(rc p) -> p rc", p=128))
            bv_bc = p1.tile([128, RL], F32)
            nc.sync.dma_start(
                out=bv_bc,
                in_=bass.AP(tensor=d["bv"].tensor, offset=d["bv"].offset,
                            ap=[[0, 128]] + list(d["bv"].ap)))
            ck_sb = p1.tile([128, T], BF16)
            nc.sync.dma_start(out=ck_sb, in_=d["ck"])
            sk_sb = p1.tile([128, T], BF16)
            nc.sync.dma_start(out=sk_sb, in_=d["sk"])
            # non-phase-1 loads issued last so they don't delay QKV
            nc.sync.dma_start(out=mask_sb, in_=d["mask"])
            nc.sync.dma_start(out=wp_sb,
                              in_=d["wp"].rearrange("(dc p) c -> p dc c", p=128))

            # ---- QKV (q/k ab-layout) + V ----
            q_ab = [p1.tile([128, T], BF16, tag=f"qab{rc}", name=f"qab{rc}")
                    for rc in range(2)]
            k_ab = [p1.tile([128, T], BF16, tag=f"kab{rc}", name=f"kab{rc}")
                    for rc in range(2)]
            g = 0  # rotate accumulation order so groups finish staggered
            for (w_sb, b_sb, dst) in ((wq_sb, bq_sb, q_ab), (wk_sb, bk_sb, k_ab)):
                for rc in range(2):
                    for tc_ in range(NT):
                        ps = ps_qkv.tile([128, 512], F32, tag="qkv", name="psqkv")
                        order = [(i + g) % CCH for i in range(CCH)]
                        for i, cc in enumerate(order):
                            nc.tensor.matmul(
                                ps,
                                w_sb[:, cc, rc * 128:(rc + 1) * 128],
                                x_sb[:, cc, tc_ * 512:(tc_ + 1) * 512],
                                start=(i == 0), stop=(i == CCH - 1))
                        nc.vector.tensor_scalar_add(
                            dst[rc][:, tc_ * 512:(tc_ + 1) * 512],
                            ps, b_sb[:, rc:rc + 1])
                        g += 1

            qr_ab = [p1.tile([128, T], BF16, tag=f"qr{i}", name=f"qr{i}")
                     for i in range(2)]
            kr_ab = [p1.tile([128, T], BF16, tag=f"kr{i}", name=f"kr{i}")
                     for i in range(2)]

            def _rope(ab, cos_sb, sin_sb, dst):
                # top = a*cos - b*sin ; bot = a*sin + b*cos
                t1 = p1.tile([128, T], BF16, tag="t1", name="t1", bufs=2)
                t2 = p1.tile([128, T], BF16, tag="t2", name="t2", bufs=2)
                nc.vector.tensor_mul(t1, ab[0], cos_sb)
                nc.vector.tensor_mul(t2, ab[1], sin_sb)
                nc.vector.tensor_sub(dst[0], t1, t2)
                t3 = p1.tile([128, T], BF16, tag="t1", name="t3", bufs=2)
                t4 = p1.tile([128, T], BF16, tag="t2", name="t4", bufs=2)
                nc.vector.tensor_mul(t3, ab[0], sin_sb)
                nc.vector.tensor_mul(t4, ab[1], cos_sb)
                nc.vector.tensor_add(dst[1], t3, t4)

            def _permute(src, dst):
                for pair in range(2):
                    for hh in range(2):
                        h = pair * 2 + hh
                        nc.sync.dma_start(
                            out=dst[pair][hh * 64:hh * 64 + 32, :],
                            in_=src[0][h * 32:(h + 1) * 32, :])
                        nc.sync.dma_start(
                            out=dst[pair][hh * 64 + 32:hh * 64 + 64, :],
                            in_=src[1][h * 32:(h + 1) * 32, :])

            # rope+permute(q) emitted before V so DVE/DMA overlap V's matmuls
            # (score scale is pre-folded into wq/bq on the host; rope is linear)
            _rope(q_ab, ck_sb, sk_sb, qr_ab)
            _permute(qr_ab, qr)

            for kc in range(TCH):
                ps = ps_qkv.tile([128, 512], F32, tag="qkv", name="psqkv")
                psv = ps[:, 0:RL]
                order = [(i + g) % CCH for i in range(CCH)]
                for i, cc in enumerate(order):
                    nc.tensor.matmul(
                        psv,
                        x_sb[:, cc, kc * 128:(kc + 1) * 128],
                        wv_sb[:, cc, :],
                        start=(i == 0), stop=(i == CCH - 1))
                g += 1
                nc.vector.scalar_tensor_tensor(
                    out=v_sb[:, kc, :, 0:64],
                    in0=psv.rearrange("p (h dd) -> p h dd", h=HPC),
                    scalar=0.0,
                    in1=bv_bc.rearrange("p (h dd) -> p h dd", h=HPC),
                    op0=mybir.AluOpType.add,
                    op1=mybir.AluOpType.add)

            _rope(k_ab, ck_sb, sk_sb, kr_ab)
            _permute(kr_ab, kr)

        # ========== attention: head pairs, packed scores, k-outer ==========
        # Heads 2p (rows 0:64) and 2p+1 (rows 64:128) of qr/kr[p] issue
        # back-to-back score matmuls at row tile_positions (0,0)/(64,0) so
        # they run concurrently in the PE array. Head A's PV accumulates in
        # pass 1; head B's probs are cached in SBUF and consumed by a
        # PV-only pass 2 (PSUM only fits one head's 4 accumulators).
        oT = [qkpool.tile([128, T], BF16, tag=f"oT{i}", name=f"oT{i}") for i in range(2)]

        def _evac(pair, hh, h, pv, tc_):
            rec = spool.tile([1, 512], F32, tag="rec", name="rec")
            nc.vector.reciprocal(rec, pv[tc_][64:65, :])
            # bounce through DRAM to broadcast across partitions
            rrow = d["recs"][h * NT + tc_:h * NT + tc_ + 1, :]
            nc.sync.dma_start(out=rrow, in_=rec)
            rbc = spool.tile([64, 512], F32, tag="rbc", name="rbc")
            nc.sync.dma_start(
                out=rbc,
                in_=bass.AP(tensor=rrow.tensor, offset=rrow.offset,
                            ap=[[0, 64], [1, 512]]))
            nc.vector.scalar_tensor_tensor(
                out=oT[pair][hh * 64:(hh + 1) * 64,
                             tc_ * 512:(tc_ + 1) * 512],
                in0=pv[tc_][0:64, :],
                scalar=0.0,
                in1=rbc,
                op0=mybir.AluOpType.add,
                op1=mybir.AluOpType.mult)

        def _pv_kb(pv, kb, h, at, hi=NT):
            k0 = kb * 128
            for tc_ in range(kb // 4, hi):
                lo = max(tc_ * 512, k0)
                nc.tensor.matmul(
                    pv[tc_][:, lo - tc_ * 512:512],
                    v_sb[:, kb, h, :],
                    at[:, lo - k0:(tc_ + 1) * 512 - k0],
                    start=(kb == 0), stop=(kb == 4 * tc_ + 3))

        _cm = contextlib.ExitStack()
        with (
            tc.tile_pool(name="ps_pv", bufs=1, space="PSUM") as ps_pv,
            tc.tile_pool(name="atp", bufs=1) as bpool,
            _cm,
        ):
            ps_sc = _cm.enter_context(
                tc.tile_pool(name="ps_sc", bufs=2, space="PSUM"))
            ps_proj = None
            for pair in range(2):
                hA, hB = 2 * pair, 2 * pair + 1
                qA, kA = qr[pair][0:64, :], kr[pair][0:64, :]
                qB, kB = qr[pair][64:128, :], kr[pair][64:128, :]
                pv = [ps_pv.tile([65, 512], F32, tag=f"pv{i}", name=f"pv{i}")
                      for i in range(NT)]
                ats = []
                for kb in range(KB):
                    k0 = kb * 128
                    # both heads' probs in one tile: [:, 0, :] = A, [:, 1, :] = B
                    at = bpool.tile([128, 2, T - k0], BF16, tag=f"at{kb}",
                                    name=f"at{kb}", bufs=2)
                    for tc_ in range(kb // 4, NT):
                        # A in bank 0, B in bank 1 of one psum tile
                        ps = ps_sc.tile([128, 2, 512], F32, tag="sc",
                                        name="ps_sc")
                        off_r = max(0, k0 - tc_ * 512)
                        # skip columns left of the diagonal (always masked)
                        qsl = slice(tc_ * 512 + off_r, (tc_ + 1) * 512)
                        nc.tensor.matmul(ps[:, 0, off_r:512],
                                         kA[:, k0:k0 + 128],
                                         qA[:, qsl], start=True, stop=True)
                        nc.tensor.matmul(ps[:, 1, off_r:512],
                                         kB[:, k0:k0 + 128],
                                         qB[:, qsl], start=True, stop=True)
                        pos = tc_ * 512 + off_r - k0
                        w = 512 - off_r
                        nc.scalar.activation(
                            out=at[:, :, pos:pos + w],
                            in_=ps[:, :, off_r:512],
                            func=mybir.ActivationFunctionType.Exp)
                    nc.vector.tensor_mul(at[:, 0, 0:128], at[:, 0, 0:128],
                                         mask_sb)
                    nc.vector.tensor_mul(at[:, 1, 0:128], at[:, 1, 0:128],
                                         mask_sb)
                    _pv_kb(pv, kb, hA, at[:, 0, :])
                    if kb % 4 == 3:
                        _evac(pair, 0, hA, pv, (kb - 3) // 4)
                    ats.append(at)
                if pair == 1:
                    # scores psum no longer needed; free its 4 banks for proj
                    _cm.close()
                    ps_proj = _cm2 = tc.tile_pool(name="ps_proj", bufs=4,
                                                  space="PSUM")
                    ps_proj = ps_proj.__enter__()
                # pass 2: head B PV from cached probs
                pv = [ps_pv.tile([65, 512], F32, tag=f"pv{i}", name=f"pvb{i}")
                      for i in range(NT)]
                for kb in range(KB):
                    _pv_kb(pv, kb, hB, ats[kb][:, 1, :])
                    if kb % 4 == 3:
                        _evac(pair, 1, hB, pv, (kb - 3) // 4)

            # ================= proj partial (sc banks reused) =============
            for t16 in range(TCH):
                o_sb = opool.tile([128, C], BF16, tag="osb")
                for half in range(2):
                    ps = ps_proj.tile([128, 512], F32, tag="proj")
                    for dc in range(2):
                        nc.tensor.matmul(
                            ps,
                            oT[dc][:, t16 * 128:(t16 + 1) * 128],
                            wp_sb[:, dc, half * 512:(half + 1) * 512],
                            start=(dc == 0), stop=(dc == 1))
                    if half == 0:
                        nc.vector.tensor_copy(
                            o_sb[:, half * 512:(half + 1) * 512], ps)
                    else:
                        nc.scalar.copy(
                            o_sb[:, half * 512:(half + 1) * 512], ps)
                nc.sync.dma_start(out=d["out"][t16 * 128:(t16 + 1) * 128, :],
                                  in_=o_sb)

            _cm2.__exit__(None, None, None)


def _host_prep(hidden_states, cos, sin, qkv_w, qkv_b, proj_w):
    cos_rep = np.tile(np.ascontiguousarray(cos.T), (HPC, 1))
    sin_rep = np.tile(np.ascontiguousarray(sin.T), (HPC, 1))
    ck = cos_rep.astype(BF)
    sk = sin_rep.astype(BF)
    mask01 = (np.arange(128)[:, None] <= np.arange(128)[None, :]).astype(BF)

    in_maps = []
    for c in range(NCORES):
        b = c // CORES_PER_B
        h0 = (c % CORES_PER_B) * HPC
        heads = list(range(h0, h0 + HPC))
        ev = [h * D + 2 * j for h in heads for j in range(D // 2)]
        od = [h * D + 2 * j + 1 for h in heads for j in range(D // 2)]
        perm = ev + od
        vrows = [h * D + dd for h in heads for dd in range(D)]
        in_maps.append(dict(
            xT=np.ascontiguousarray(hidden_states[b].T).astype(BF),
            wq_t=np.ascontiguousarray(
                qkv_w[0 * H * D:1 * H * D][perm].T * SCALE).astype(BF),
            wk_t=np.ascontiguousarray(qkv_w[1 * H * D:2 * H * D][perm].T).astype(BF),
            wv_t=np.ascontiguousarray(qkv_w[2 * H * D:3 * H * D][vrows].T).astype(BF),
            wproj_t=np.ascontiguousarray(proj_w[:, vrows].T).astype(BF),
            bq=np.ascontiguousarray(qkv_b[0 * H * D:1 * H * D][perm] * SCALE),
            bk=np.ascontiguousarray(qkv_b[1 * H * D:2 * H * D][perm]),
            bv=np.ascontiguousarray(qkv_b[2 * H * D:3 * H * D][vrows]),
            cos_k=ck, sin_k=sk, mask01=mask01,
        ))
    return in_maps


def kernel(hidden_states, cos, sin, qkv_w, qkv_b, proj_w, proj_b):
    hidden_states = np.asarray(hidden_states, dtype=np.float32)
    cos = np.asarray(cos, dtype=np.float32)
    sin = np.asarray(sin, dtype=np.float32)
    qkv_w = np.asarray(qkv_w, dtype=np.float32)
    qkv_b = np.asarray(qkv_b, dtype=np.float32)
    proj_w = np.asarray(proj_w, dtype=np.float32)
    proj_b = np.asarray(proj_b, dtype=np.float32)

    if "nc" not in _compiled:
        _compiled["nc"] = _build()
    nc = _compiled["nc"]

    in_maps = _host_prep(hidden_states, cos, sin, qkv_w, qkv_b, proj_w)
    res = run_bass_kernel_spmd(nc, in_maps, core_ids=list(range(NCORES)))
    outs = [np.asarray(res.results[c]["out"], dtype=np.float32)
            for c in range(NCORES)]
    final = np.empty((B, T, C), np.float32)
    for b in range(B):
        acc = outs[b * CORES_PER_B].copy()
        for i in range(1, CORES_PER_B):
            acc += outs[b * CORES_PER_B + i]
        final[b] = acc + proj_b[None, :]
    return final

